# revision 9
# baseline (speedup 1.0000x reference)
"""TRN2 Bass kernel for nn_AttnPlainNet (gnn_message_passing).

Math (C=1 collapses everything):
  l2norm over C=1  -> u = sign(x), sgn_nb = sign(neighbor)
  att weights      -> watt[b,n] = softmax_n(s_x[b]*s_y[b,n])
  v[b,f] = sum_n watt*sgn_nb ; w = u*v
  fadj[a,e] = u_a u_e S(w_a+w_e) / (d_e + eps),  S(t)=sign(t)sqrt|t|,
  d_e = sum_a R(w_a+w_e), R(t)=sqrt|t|
  z1[k] = u_k t_k/(d_k+eps), t_k = sum_f S(w_f+w_k)

Key trick: S and R are evaluated through a fixed rank-RK separable
expansion  phi(u+v) ~= sum_m P_m(u) Q_m(v)  built from an SVD of the
function on a grid.  Each batch's w is rescaled to [-1,1] (sqrt scale
invariance cancels in t/d and S/d), the P/Q basis values are fetched
with one 512-row dma_gather per batch (table row = [PS|QS|PR|QR] fp16),
and all of t, d and the layer-2 bilinear form become tiny PE matmuls:
  cS = colsum P(w),  t = QS(w)^T cS,  d = QR(w)^T cR,
  A^T = PS(w)_fmajor^T ptil,  z2 = QS(w)^T A^T * u/(d+eps)
BN1/BN2 stats are the same 2-float / 16x17 all-reduces as before.
Sharding: pure data-parallel, 32 batches per core, 8 cores.
"""
from contextlib import ExitStack

import numpy as np

import concourse.bass as bass
import concourse.mybir as mybir
import concourse.tile as tile
from concourse import bacc
from concourse.bass_utils import run_bass_kernel_spmd
from concourse.masks import make_identity

# Keep Ln+Exp resident in natural_log_exp_and_others (no table swaps).
_orig_get_tables = bacc.get_activation_tables


def _patched_get_tables(arch):
    tabs = dict(_orig_get_tables(arch))
    for name in ("natural_log", "exp_and_others", "exp_and_friends"):
        if name in tabs:
            tabs[name] = set()
    return tabs


bacc.get_activation_tables = _patched_get_tables

AF = mybir.ActivationFunctionType
ALU = mybir.AluOpType
F32 = mybir.dt.float32
F16 = mybir.dt.float16
I16 = mybir.dt.int16
U16 = mybir.dt.uint16

B, N, F, H, NCLS = 256, 32, 512, 16, 64
NCORES = 8
BL = B // NCORES          # 32 local batches
FC = 4                    # f/k chunks of 128
P = 128
EPS_ROW = 1e-7
EPS_BN = 1e-5
NK = float(B * F)         # BN normalizer (global)

# low-rank table parameters
RK = 48                   # rank per kernel (padded to 64 cols in the row)
EW = 256                  # table row: [PS(48)+pad16|QS|PR|QR] fp16 = 512B
GM = 4096                 # grid size
GH = 2.0 / (GM - 8)
G0 = -1.0 - 3.5 * GH

_CACHE = {}


def _bc_ap(handle_ap, ap):
    return bass.AP(tensor=handle_ap.tensor, offset=handle_ap.offset, ap=ap)


def build_tables():
    """Rank-RK separable expansion of sgnroot and sqrt-abs on the grid."""
    g = (G0 + GH * np.arange(GM)).astype(np.float64)

    def build(phi):
        rng = np.random.default_rng(12345)
        PHI = phi(g[:, None] + g[None, :])          # [GM, GM] symmetric
        G = rng.standard_normal((GM, RK + 16))
        Y = PHI @ (PHI @ (PHI @ G))                 # one power iteration
        Qo, _ = np.linalg.qr(Y)
        Bm = Qo.T @ PHI
        U2, s, Vt = np.linalg.svd(Bm, full_matrices=False)
        U = Qo @ U2[:, :RK]
        Pm = U * s[:RK]
        Qm = Vt[:RK, :].T
        # balance magnitudes for fp16
        c = np.sqrt(
            (np.abs(Pm).max(axis=0) + 1e-30) / (np.abs(Qm).max(axis=0) + 1e-30))
        Pm = Pm / c
        Qm = Qm * c
        return Pm.astype(np.float32), Qm.astype(np.float32)

    PS, QS = build(lambda t: np.sign(t) * np.sqrt(np.abs(t)))
    PR, QR = build(lambda t: np.sqrt(np.abs(t)))
    tab = np.zeros((GM, EW), np.float16)
    tab[:, 0:RK] = PS.astype(np.float16)
    tab[:, 64:64 + RK] = QS.astype(np.float16)
    tab[:, 128:128 + RK] = PR.astype(np.float16)
    tab[:, 192:192 + RK] = QR.astype(np.float16)
    return tab


def build_program(no_cc=False):
    nc = bacc.Bacc("TRN2", num_devices=NCORES)

    # ---- I/O -------------------------------------------------------------
    x_l = nc.dram_tensor("x_l", [BL, F], F32, kind="ExternalInput")
    nb_l = nc.dram_tensor("nb_l", [BL * N, F], F32, kind="ExternalInput")
    att1 = nc.dram_tensor("att1", [1, F], F32, kind="ExternalInput")
    att2 = nc.dram_tensor("att2", [1, F], F32, kind="ExternalInput")
    tabd = nc.dram_tensor("tabd", [GM, EW], F16, kind="ExternalInput")
    w1c = nc.dram_tensor("w1c", [H, 1], F32, kind="ExternalInput")
    b1 = nc.dram_tensor("b1", [H, 1], F32, kind="ExternalInput")
    g1 = nc.dram_tensor("g1", [H, 1], F32, kind="ExternalInput")
    be1 = nc.dram_tensor("be1", [H, 1], F32, kind="ExternalInput")
    w2 = nc.dram_tensor("w2", [H, H], F32, kind="ExternalInput")
    w2t = nc.dram_tensor("w2t", [H, H], F32, kind="ExternalInput")
    b2 = nc.dram_tensor("b2", [H, 1], F32, kind="ExternalInput")
    g2 = nc.dram_tensor("g2", [H, 1], F32, kind="ExternalInput")
    be2 = nc.dram_tensor("be2", [H, 1], F32, kind="ExternalInput")
    wct = nc.dram_tensor("wct", [H * F, NCLS], F16, kind="ExternalInput")
    bc = nc.dram_tensor("bc", [1, NCLS], F32, kind="ExternalInput")
    out_l = nc.dram_tensor("out_l", [BL, NCLS], F32, kind="ExternalOutput")
    import os
    DBG = os.environ.get("KDBG") == "1"
    if DBG:
        dbg_gidx = nc.dram_tensor("dbg_gidx", [P, BL, 32], I16, kind="ExternalOutput")
        dbg_got = nc.dram_tensor("dbg_got", [P, 2, F], F16, kind="ExternalOutput")
        dbg_cs = nc.dram_tensor("dbg_cs", [P, BL, 2], F16, kind="ExternalOutput")
        dbg_z1 = nc.dram_tensor("dbg_z1", [P, FC, BL], F32, kind="ExternalOutput")
        dbg_urd = nc.dram_tensor("dbg_urd", [P, FC, BL], F32, kind="ExternalOutput")
        dbg_z2 = nc.dram_tensor("dbg_z2", [P, FC, BL, H], F16, kind="ExternalOutput")
        dbg_w = nc.dram_tensor("dbg_w", [P, 16], F32, kind="ExternalOutput")

    with tile.TileContext(nc) as tc, ExitStack() as ctx:
        sg = ctx.enter_context(tc.tile_pool(name="singles", bufs=1))
        wk = ctx.enter_context(tc.tile_pool(name="work", bufs=2))
        t2 = ctx.enter_context(tc.tile_pool(name="t2", bufs=2))
        big = ctx.enter_context(tc.tile_pool(name="big", bufs=1))
        st1ctx = ExitStack()
        s1 = st1ctx.enter_context(tc.tile_pool(name="stage1", bufs=1))
        dr = ctx.enter_context(tc.tile_pool(name="dram", bufs=1, space="DRAM"))
        ps = ctx.enter_context(tc.tile_pool(name="psmall", bufs=2, space="PSUM"))
        pj = ctx.enter_context(tc.tile_pool(name="pj", bufs=2, space="PSUM"))
        pm2 = ctx.enter_context(tc.tile_pool(name="pm2", bufs=1, space="PSUM"))
        pq = ctx.enter_context(tc.tile_pool(name="pq", bufs=1, space="PSUM"))

        V, S, G = nc.vector, nc.scalar, nc.gpsimd
        TE = nc.tensor

        # ---- constants ---------------------------------------------------
        i32 = sg.tile([32, 32], F32)
        make_identity(nc, i32[:])
        i16 = sg.tile([16, 16], F32)
        make_identity(nc, i16[:])
        i16h = sg.tile([16, 16], F16)
        make_identity(nc, i16h[:])
        i128 = sg.tile([P, P], F32)
        make_identity(nc, i128[:])
        i128h = sg.tile([P, P], F16)
        make_identity(nc, i128h[:])
        i4 = sg.tile([4, 4], F32)
        make_identity(nc, i4[:])
        epsb = sg.tile([H, 1], F32)
        V.memset(epsb[:], EPS_BN)
        ones128 = sg.tile([P, 1], F32)
        V.memset(ones128[:], 1.0)
        ones128h = sg.tile([P, 1], F16)
        V.memset(ones128h[:], 1.0)
        onesrow = sg.tile([1, P], F32)
        V.memset(onesrow[:], 1.0)
        blkones = sg.tile([P, 4], F32)
        V.memset(blkones[:], 0.0)
        for a in range(4):
            V.memset(blkones[32 * a:32 * a + 32, a:a + 1], 1.0)
        # tiled identity [16, 128]: id16t[j, p] = (p % 16 == j)
        id16t = sg.tile([16, P], F32)
        for gidx8 in range(8):
            V.tensor_copy(id16t[:, 16 * gidx8:16 * gidx8 + 16], i16[:])

        # broadcast att vectors
        att1_b = s1.tile([32, F], F32)
        nc.sync.dma_start(att1_b[:], _bc_ap(att1[:], [[0, 32], [1, F]]))
        att2_b = s1.tile([P, F], F32)
        nc.sync.dma_start(att2_b[:], _bc_ap(att2[:], [[0, P], [1, F]]))

        # WcT tiles [128, 64jc, 64n] fp16
        wct_sb = sg.tile([P, 64, NCLS], F16)
        nc.sync.dma_start(wct_sb[:], wct[:].rearrange("(jc p) n -> p jc n", p=P))
        bc_rep = sg.tile([8, NCLS], F32)
        nc.sync.dma_start(bc_rep[:], _bc_ap(bc[:], [[0, 8], [1, NCLS]]))

        # per-channel weights [16,1]
        w1s = sg.tile([H, 1], F32)
        nc.sync.dma_start(w1s[:], w1c[:])
        b1s = sg.tile([H, 1], F32)
        nc.sync.dma_start(b1s[:], b1[:])
        g1s = sg.tile([H, 1], F32)
        nc.sync.dma_start(g1s[:], g1[:])
        be1s = sg.tile([H, 1], F32)
        nc.sync.dma_start(be1s[:], be1[:])
        b2s = sg.tile([H, 1], F32)
        nc.sync.dma_start(b2s[:], b2[:])
        g2s = sg.tile([H, 1], F32)
        nc.sync.dma_start(g2s[:], g2[:])
        be2s = sg.tile([H, 1], F32)
        nc.sync.dma_start(be2s[:], be2[:])
        w2s = sg.tile([H, H], F32)
        nc.sync.dma_start(w2s[:], w2[:])
        w2ts = sg.tile([H, H], F32)
        nc.sync.dma_start(w2ts[:], w2t[:])

        # ---- stage 0: x -> u, s_x ---------------------------------------
        xsb = wk.tile([P, F], F32, tag="nbt")
        nc.sync.dma_start(xsb[0:BL, :], x_l[:])
        u32 = sg.tile([BL, F], F32)
        S.activation(u32[:], xsb[0:BL, :], AF.Sign)
        sx_col = sg.tile([BL, 1], F32)
        V.scalar_tensor_tensor(xsb[0:BL, :], u32[:], 0.0, att1_b[:],
                               ALU.bypass, ALU.mult, accum_out=sx_col[:])

        # u transpose (f-major)
        p_tu = ps.tile([P, P], F32, tag="sm")
        for c in range(FC):
            TE.transpose(p_tu[:, 32 * c:32 * c + 32],
                         u32[:, P * c:P * c + P], i32[:])
        uT = sg.tile([P, P], F32)          # [p, 32c+b]
        V.tensor_copy(uT[:], p_tu[:])
        uT16 = sg.tile([P, P], F16)
        V.tensor_copy(uT16[:], uT[:])

        sx_d = dr.tile([BL], F32)
        nc.sync.dma_start(sx_d[:], sx_col[:].rearrange("b one -> (b one)"))
        sx_rep = sg.tile([P, 8], F32)
        for a in range(4):
            nc.sync.dma_start(sx_rep[32 * a:32 * a + 32, :],
                        bass.AP(tensor=sx_d[:].tensor,
                                offset=sx_d[:].offset + a,
                                ap=[[0, 32], [4, 8]]))

        # ---- stage 1 per j: sgn, s_y, softmax, v, w~, gather indices -----
        gidx = sg.tile([P, BL, 32], I16)   # gather indices, replicated x8
        for j in range(8):
            nbt = wk.tile([P, F], F32, tag="nbt")
            nc.sync.dma_start(nbt[:], nb_l[:].rearrange("(j p) f -> j p f", p=P)[j])
            sgn = wk.tile([P, F], F32, tag="sgn")
            S.activation(sgn[:], nbt[:], AF.Sign)
            sy = wk.tile([P, 1], F32, tag="sy")
            V.scalar_tensor_tensor(nbt[:], sgn[:], 0.0, att2_b[:],
                                   ALU.bypass, ALU.mult, accum_out=sy[:])
            lcol = wk.tile([P, 1], F32, tag="lcol")
            V.tensor_tensor(lcol[:], sy[:], sx_rep[:, j:j + 1], ALU.mult)
            ecol = wk.tile([P, 1], F32, tag="ecol")
            S.activation(ecol[:], lcol[:], AF.Exp)
            p_dn = ps.tile([4, 1], F32, tag="sm")
            TE.matmul(p_dn[:], blkones[:], ecol[:], start=True, stop=True)
            rdn = wk.tile([4, 1], F32, tag="rdn")
            V.reciprocal(rdn[:], p_dn[:])
            wd4 = wk.tile([P, 4], F32, tag="wd")
            V.tensor_tensor(wd4[:], ecol[:].to_broadcast([P, 4]),
                            blkones[:], ALU.mult)
            p_vj = ps.tile([4, F], F32, tag="sm")
            TE.matmul(p_vj[:], wd4[:], sgn[:], start=True, stop=True)
            u_j = wk.tile([4, F], F32, tag="uj")
            nc.sync.dma_start(u_j[:], u32[4 * j:4 * j + 4, :])
            w_j = wk.tile([4, F], F32, tag="wj")
            V.tensor_scalar(w_j[:], p_vj[:], rdn[:], None, ALU.mult)
            V.tensor_tensor(w_j[:], w_j[:], u_j[:], ALU.mult)
            # normalize per b: w~ = w / max|w|
            smax = wk.tile([4, 1], F32, tag="smax")
            V.tensor_reduce(smax[:], w_j[:], mybir.AxisListType.X, ALU.max,
                            apply_absolute_value=True)
            rsc = wk.tile([4, 1], F32, tag="rsc")
            V.reciprocal(rsc[:], smax[:])
            V.tensor_scalar(w_j[:], w_j[:], rsc[:], None, ALU.mult)
            # transpose w~ -> [128, 4c, 4b]
            p_wt = ps.tile([P, 4, 4], F32, tag="sm")
            for c in range(FC):
                TE.transpose(p_wt[:, c, :], w_j[:, P * c:P * c + P], i4[:])
            if DBG and j == 0:
                wdbg = wk.tile([P, 16], F32, tag="wdbg")
                V.tensor_copy(wdbg[:], p_wt[:].rearrange("p c b -> p (c b)"))
                nc.sync.dma_start(dbg_w[:], wdbg[:])
            # idx affine (values are integral grid indices in f32)
            idxf = wk.tile([P, 16], F32, tag="idxf")
            V.tensor_scalar(idxf[:], p_wt[:].rearrange("p c b -> p (c b)"),
                            1.0 / GH, -G0 / GH, ALU.mult, ALU.add)
            # fold k%16 onto partitions: [128,16] -> [16,128] -> 8x [16,16]T
            p_it = ps.tile([16, P], F32, tag="sm")
            TE.transpose(p_it[:], idxf[:], i128[:])
            idxT = wk.tile([16, P], F32, tag="idxT")   # [cb, k_low]
            V.tensor_copy(idxT[:], p_it[:])
            p_fold = ps.tile([16, 8, 16], F32, tag="sm")
            for kh in range(8):
                TE.transpose(p_fold[:, kh, :], idxT[:, 16 * kh:16 * kh + 16],
                             i16[:])
            foldsb = wk.tile([16, 8, 4, 4], F32, tag="fold")  # [p16, kh, c, b]
            V.tensor_copy(foldsb[:], p_fold[:])
            # replicate to 128 partitions; moving free order (b, c, kh)
            fap = foldsb[:]
            mov = bass.AP(tensor=fap.tensor, offset=fap.offset,
                          ap=[fap.ap[0], [1, 4], [4, 4], [16, 8]])
            p_rep = ps.tile([P, 4, 4, 8], F32, tag="sm")   # [p, b, c, kh]
            TE.matmul(p_rep[:], id16t[:], mov, start=True, stop=True)
            V.tensor_copy(gidx[:, 4 * j:4 * j + 4, :],
                          p_rep[:].rearrange("p b c h -> p b (c h)"))
        st1ctx.close()

        # ---- stage 2a: gathers, PS f-major, cS/cR, t/d, z1 ---------------
        got = big.tile([P, BL, 2, F], F16, tag="got")
        psfm = sg.tile([P, BL, FC, 64], F16)
        psfmR = sg.tile([P, BL, FC, 64], F16)
        cs16 = sg.tile([P, BL, 2], F16)    # [64:128]: col0 = cS, col1 = cR
        urdT = sg.tile([P, FC, BL], F32)
        z1T = sg.tile([P, FC, BL], F32)
        accs = sg.tile([P, 8, 2], F32)

        for b in range(BL):
            G.dma_gather(got[:, b, :, :], tabd[:], gidx[:, b, :],
                         F, F, EW, transpose=True)
        for b in range(BL):
            for kc in range(FC):
                nc.sync.dma_start_transpose(
                    psfm[:, b, kc, :], got[0:64, b, 0, P * kc:P * kc + P])
                nc.sync.dma_start_transpose(
                    psfmR[:, b, kc, :], got[0:64, b, 1, P * kc:P * kc + P])

        for j in range(8):
            p_td = pj.tile([P, 4, FC, 2], F32, tag="td")
            for bb in range(4):
                b = 4 * j + bb
                # cS column: stationary psfm chunks, moving ones
                p_c = ps.tile([P, 1], F32, tag="sm")
                for fc in range(FC):
                    TE.matmul(p_c[64:128, :], psfm[:, b, fc, :], ones128h[:],
                              start=(fc == 0), stop=(fc == FC - 1))
                V.tensor_copy(cs16[64:128, b, 0:1], p_c[64:128, :])
                # cR column via PE colsum of PR f-major
                p_cr = ps.tile([P, 1], F32, tag="sm")
                for fc in range(FC):
                    TE.matmul(p_cr[64:128, :], psfmR[:, b, fc, :], ones128h[:],
                              start=(fc == 0), stop=(fc == FC - 1))
                V.tensor_copy(cs16[64:128, b, 1:2], p_cr[64:128, :])
                # t/d matmuls
                for kc in range(FC):
                    TE.matmul(p_td[:, bb, kc, 0:1],
                              got[64:128, b, 0, P * kc:P * kc + P],
                              cs16[64:128, b, 0:1], start=True, stop=True)
                    TE.matmul(p_td[:, bb, kc, 1:2],
                              got[64:128, b, 1, P * kc:P * kc + P],
                              cs16[64:128, b, 1:2], start=True, stop=True)
            # z1 = u*t/(d+eps) for the 4 b's of this j
            rd = wk.tile([P, 4, FC], F32, tag="rd")
            V.tensor_scalar(rd[:], p_td[:, :, :, 1], EPS_ROW, None, ALU.add)
            V.reciprocal(rd[:], rd[:])
            uslc = bass.AP(tensor=uT[:].tensor, offset=uT[:].offset + 4 * j,
                           ap=[[P, P], [1, 4], [32, FC]])   # [p, b, c]
            V.tensor_tensor(rd[:], rd[:], uslc, ALU.mult)   # u/(d+eps)
            for bb in range(4):
                V.tensor_copy(urdT[:, :, 4 * j + bb], rd[:, bb, :])
            zj = wk.tile([P, 4, FC], F32, tag="zj")
            V.tensor_tensor(zj[:], p_td[:, :, :, 0], rd[:], ALU.mult)
            for bb in range(4):
                V.tensor_copy(z1T[:, :, 4 * j + bb], zj[:, bb, :])
            # BN1 partial sums
            V.tensor_scalar(zj[:], zj[:], 1.0, 0.0, ALU.mult, ALU.add,
                            accum_out=accs[:, j, 0:1])
            zsq = wk.tile([P, 4, FC], F32, tag="zsq")
            V.scalar_tensor_tensor(zsq[:], zj[:], 0.0, zj[:],
                                   ALU.bypass, ALU.mult,
                                   accum_out=accs[:, j, 1:2])

        if DBG:
            nc.sync.dma_start(dbg_gidx[:], gidx[:])
            nc.sync.dma_start(dbg_got[:], got[:, 0, :, :])
            nc.sync.dma_start(dbg_cs[:], cs16[:])
            nc.sync.dma_start(dbg_z1[:], z1T[:])
            nc.sync.dma_start(dbg_urd[:], urdT[:])

        # ---- BN1 stats + all-reduce --------------------------------------
        rs = sg.tile([P, 2], F32)
        V.tensor_reduce(rs[:], accs[:].rearrange("p j t -> p t j"),
                        mybir.AxisListType.X, ALU.add)
        p_s = ps.tile([1, 2], F32, tag="sm")
        TE.matmul(p_s[:], ones128[:], rs[:], start=True, stop=True)
        s_loc = sg.tile([1, 2], F32)
        V.tensor_copy(s_loc[:], p_s[:])
        cc1_in = dr.tile([1, 2], F32)
        cc1_out = dr.tile([1, 2], F32)
        nc.sync.dma_start(cc1_in[:], s_loc[:])
        if no_cc:
            nc.sync.dma_start(cc1_out[:], cc1_in[:])
        else:
            G.collective_compute("AllReduce", ALU.add,
                                 replica_groups=[list(range(NCORES))],
                                 ins=[cc1_in[:].opt()],
                                 outs=[cc1_out[:].opt()])
        sg_b = sg.tile([H, 2], F32)
        nc.sync.dma_start(sg_b[:], _bc_ap(cc1_out[:], [[0, H], [1, 2]]))

        # per-channel BN1 affine params
        mz = sg.tile([H, 1], F32)
        V.tensor_scalar(mz[:], sg_b[:, 0:1], 1.0 / NK, None, ALU.mult)
        e2m = sg.tile([H, 1], F32)
        V.tensor_scalar(e2m[:], sg_b[:, 1:2], 1.0 / NK, None, ALU.mult)
        tmp = sg.tile([H, 1], F32)
        V.tensor_tensor(tmp[:], mz[:], mz[:], ALU.mult)
        varz = sg.tile([H, 1], F32)
        V.tensor_tensor(varz[:], e2m[:], tmp[:], ALU.subtract)
        w1sq = sg.tile([H, 1], F32)
        V.tensor_tensor(w1sq[:], w1s[:], w1s[:], ALU.mult)
        var1 = sg.tile([H, 1], F32)
        V.tensor_tensor(var1[:], w1sq[:], varz[:], ALU.mult)
        invsd = sg.tile([H, 1], F32)
        S.activation(invsd[:], var1[:], AF.Ln, bias=epsb[:])
        S.activation(invsd[:], invsd[:], AF.Exp, scale=-0.5)
        alpha = sg.tile([H, 1], F32)
        V.tensor_tensor(alpha[:], w1s[:], g1s[:], ALU.mult)
        V.tensor_tensor(alpha[:], alpha[:], invsd[:], ALU.mult)
        m1 = sg.tile([H, 1], F32)
        V.tensor_tensor(m1[:], w1s[:], mz[:], ALU.mult)
        V.tensor_tensor(m1[:], m1[:], b1s[:], ALU.add)
        beta = sg.tile([H, 1], F32)
        V.tensor_tensor(beta[:], b1s[:], m1[:], ALU.subtract)
        V.tensor_tensor(beta[:], beta[:], g1s[:], ALU.mult)
        V.tensor_tensor(beta[:], beta[:], invsd[:], ALU.mult)
        V.tensor_tensor(beta[:], beta[:], be1s[:], ALU.add)

        p_ab = ps.tile([1, 2 * H], F32, tag="sm")
        TE.transpose(p_ab[:, 0:H], alpha[:], i16[:])
        TE.transpose(p_ab[:, H:2 * H], beta[:], i16[:])
        ab_row = sg.tile([1, 2 * H], F32)
        V.tensor_copy(ab_row[:], p_ab[:])
        p_abb = ps.tile([P, 2 * H], F32, tag="sm")
        TE.matmul(p_abb[:, 0:H], onesrow[:], ab_row[0:1, 0:H],
                  start=True, stop=True)
        TE.matmul(p_abb[:, H:2 * H], onesrow[:], ab_row[0:1, H:2 * H],
                  start=True, stop=True)
        abb = sg.tile([P, 2 * H], F32)
        V.tensor_copy(abb[:], p_abb[:])
        alpha_b = abb[:, 0:H]
        beta_b = abb[:, H:2 * H]

        # ---- ptil = softsign(alpha*z1+beta)*u  [128, fc, b, 16] fp16 -----
        ptil = big.tile([P, FC, BL, H], F16, tag="ptil")
        sfull = t2.tile([P, FC, BL, H], F16, tag="T")
        den = t2.tile([P, FC, BL, H], F16, tag="sig")
        V.tensor_tensor(sfull[:],
                        z1T[:, :, :, None].to_broadcast([P, FC, BL, H]),
                        alpha_b[:, None, None, :].to_broadcast([P, FC, BL, H]),
                        ALU.mult)
        V.tensor_tensor(sfull[:], sfull[:],
                        beta_b[:, None, None, :].to_broadcast([P, FC, BL, H]),
                        ALU.add)
        S.activation(den[:], sfull[:], AF.Abs)
        V.tensor_scalar(den[:], den[:], 1.0, None, ALU.add)
        with nc.allow_low_precision(reason="softsign denom fp16 ok"):
            V.reciprocal(den[:], den[:])
        V.tensor_tensor(ptil[:], sfull[:], den[:], ALU.mult)
        V.tensor_tensor(ptil[:], ptil[:],
                        uT16[:].rearrange("p (c b) -> p c b", c=FC)
                        [:, :, :, None].to_broadcast([P, FC, BL, H]),
                        ALU.mult)

        # ---- stage 2b: A^T, z2 -------------------------------------------
        z2T = sg.tile([P, FC, BL, H], F16)
        at16 = sg.tile([P, BL, H], F16)   # A^T at partitions 64:112
        for j in range(8):
            for bb in range(4):
                b = 4 * j + bb
                p_at = ps.tile([P, H], F32, tag="sm")
                for fc in range(FC):
                    TE.matmul(p_at[64:128, :], psfm[:, b, fc, :],
                              ptil[:, fc, b, :],
                              start=(fc == 0), stop=(fc == FC - 1))
                V.tensor_copy(at16[64:128, b, :], p_at[64:128, :])
            p_z2 = pj.tile([P, 4, FC, H], F32, tag="td")
            for bb in range(4):
                b = 4 * j + bb
                for kc in range(FC):
                    TE.matmul(p_z2[:, bb, kc, :],
                              got[64:128, b, 0, P * kc:P * kc + P],
                              at16[64:128, b, :], start=True, stop=True)
            uslc2 = bass.AP(tensor=urdT[:].tensor,
                            offset=urdT[:].offset + 4 * j,
                            ap=[[FC * BL, P], [1, 4], [BL, FC], [0, H]])
            V.tensor_tensor(
                z2T[:, :, 4 * j:4 * j + 4, :].rearrange("p c b h -> p b c h"),
                p_z2[:], uslc2, ALU.mult)

        if DBG:
            nc.sync.dma_start(dbg_z2[:], z2T[:])

        # ---- BN2 moments + all-reduce ------------------------------------
        p_m2 = pm2.tile([H, H], F32, tag="pm2")
        p_m1 = pm2.tile([1, H], F32, tag="pm1")
        for cb in range(FC * BL):
            kc, b = divmod(cb, BL)
            TE.matmul(p_m2[:], z2T[:, kc, b, :], z2T[:, kc, b, :],
                      start=(cb == 0), stop=(cb == FC * BL - 1))
        for cb in range(FC * BL):
            kc, b = divmod(cb, BL)
            TE.matmul(p_m1[:], ones128h[:], z2T[:, kc, b, :],
                      start=(cb == 0), stop=(cb == FC * BL - 1))
        m2_sb = sg.tile([H, H], F32)
        V.tensor_copy(m2_sb[:], p_m2[:])
        m1_sb = sg.tile([1, H], F32)
        V.tensor_copy(m1_sb[:], p_m1[:])
        cc2_in = dr.tile([H + 1, H], F32)
        cc2_out = dr.tile([H + 1, H], F32)
        nc.sync.dma_start(cc2_in[0:H, :], m2_sb[:])
        nc.sync.dma_start(cc2_in[H:H + 1, :], m1_sb[:])
        if no_cc:
            nc.sync.dma_start(cc2_out[:], cc2_in[:])
        else:
            G.collective_compute("AllReduce", ALU.add,
                                 replica_groups=[list(range(NCORES))],
                                 ins=[cc2_in[:].opt()],
                                 outs=[cc2_out[:].opt()])
        m2g = sg.tile([H, H], F32)
        nc.sync.dma_start(m2g[:], cc2_out[0:H, :])
        m1_b = sg.tile([H, H], F32)
        c2ap = cc2_out[:]
        nc.sync.dma_start(m1_b[:], bass.AP(tensor=c2ap.tensor,
                                     offset=c2ap.offset + H * H,
                                     ap=[[0, H], [1, H]]))

        # ---- BN2 affine params -------------------------------------------
        p_a1 = ps.tile([H, H], F32, tag="sm")
        TE.matmul(p_a1[:], w2ts[:], m2g[:], start=True, stop=True)
        a1 = sg.tile([H, H], F32)
        V.tensor_copy(a1[:], p_a1[:])
        t16 = sg.tile([H, H], F32)
        V.tensor_tensor(t16[:], a1[:, 0:H], w2s[:], ALU.mult)
        diagq = sg.tile([H, 1], F32)
        V.reduce_sum(diagq[:], t16[:], axis=mybir.AxisListType.X)
        wm1t = sg.tile([H, H], F32)
        V.tensor_tensor(wm1t[:], w2s[:], m1_b[:], ALU.mult)
        wm1 = sg.tile([H, 1], F32)
        V.reduce_sum(wm1[:], wm1t[:], axis=mybir.AxisListType.X)
        m2o = sg.tile([H, 1], F32)
        V.tensor_scalar(m2o[:], wm1[:], 1.0 / NK, None, ALU.mult)
        V.tensor_tensor(m2o[:], m2o[:], b2s[:], ALU.add)
        eh2 = sg.tile([H, 1], F32)
        V.tensor_scalar(eh2[:], diagq[:], 1.0 / NK, None, ALU.mult)
        tb2 = sg.tile([H, 1], F32)
        V.tensor_tensor(tb2[:], b2s[:], wm1[:], ALU.mult)
        V.tensor_scalar(tb2[:], tb2[:], 2.0 / NK, None, ALU.mult)
        V.tensor_tensor(eh2[:], eh2[:], tb2[:], ALU.add)
        b2sq = sg.tile([H, 1], F32)
        V.tensor_tensor(b2sq[:], b2s[:], b2s[:], ALU.mult)
        V.tensor_tensor(eh2[:], eh2[:], b2sq[:], ALU.add)
        m2sq = sg.tile([H, 1], F32)
        V.tensor_tensor(m2sq[:], m2o[:], m2o[:], ALU.mult)
        var2 = sg.tile([H, 1], F32)
        V.tensor_tensor(var2[:], eh2[:], m2sq[:], ALU.subtract)
        invsd2 = sg.tile([H, 1], F32)
        S.activation(invsd2[:], var2[:], AF.Ln, bias=epsb[:])
        S.activation(invsd2[:], invsd2[:], AF.Exp, scale=-0.5)
        gam = sg.tile([H, 1], F32)
        V.tensor_tensor(gam[:], g2s[:], invsd2[:], ALU.mult)
        w2p = sg.tile([H, H], F16)
        V.tensor_scalar(w2p[:], w2s[:], gam[:], None, ALU.mult)
        delta = sg.tile([H, 1], F32)
        V.tensor_tensor(delta[:], b2s[:], m2o[:], ALU.subtract)
        V.tensor_tensor(delta[:], delta[:], gam[:], ALU.mult)
        V.tensor_tensor(delta[:], delta[:], be2s[:], ALU.add)

        p_w2p = ps.tile([H, H], F16, tag="sm")
        TE.transpose(p_w2p[:], w2p[:], i16h[:])
        w2pt = sg.tile([H, H], F16)
        V.tensor_copy(w2pt[:], p_w2p[:])
        bd = sg.tile([P, P], F16)
        V.memset(bd[:], 0.0)
        w2pt_d = dr.tile([H, H], F16)
        nc.sync.dma_start(w2pt_d[:], w2pt[:])
        for i in range(8):
            nc.sync.dma_start(bd[16 * i:16 * i + 16, 16 * i:16 * i + 16],
                        w2pt_d[:])
        i16big = sg.tile([H, P], F32)
        for i in range(8):
            V.tensor_copy(i16big[:, H * i:H * i + H], i16[:])
        p_dl = ps.tile([P, 1], F32, tag="sm")
        TE.matmul(p_dl[:], i16big[:], delta[:], start=True, stop=True)
        dl_rep = sg.tile([P, 1], F32)
        V.tensor_copy(dl_rep[:], p_dl[:])

        # ---- q phase + classifier ----
        qt_all = big.tile([P, 4, FC, P], F16, tag="qt")
        qs_all = t2.tile([P, 4, F], F16, tag="T")
        for g in range(4):
            pp = pq if g % 2 == 0 else pm2
            p_z2c = pp.tile([P, F], F16, tag="pm2" if g % 2 else "pz2c",
                            name=f"pz2c{g}")
            for kc in range(FC):
                TE.transpose(p_z2c[:, P * kc:P * kc + P],
                             z2T[:, kc, 8 * g:8 * g + 8, :], i128h[:])
            z2c = wk.tile([P, F], F16, tag="z2c")
            V.tensor_copy(z2c[:], p_z2c[:])
            p_q = pp.tile([P, F], F32, tag="pm1" if g % 2 else "pqm",
                          name=f"pqm{g}")
            TE.matmul(p_q[:], bd[:], z2c[:], start=True, stop=True)
            V.tensor_scalar(qs_all[:, g, :], p_q[:], dl_rep[:], None, ALU.add)
        rq_all = t2.tile([P, 4, F], F16, tag="sig")
        q8_all = t2.tile([P, 4, F], F16, tag="r")
        for h in range(2):
            sl = slice(h * 2, (h + 1) * 2)
            S.activation(rq_all[:, sl, :], qs_all[:, sl, :], AF.Abs)
            S.activation(rq_all[:, sl, :], rq_all[:, sl, :], AF.Ln, bias=1.0)
            S.activation(rq_all[:, sl, :], rq_all[:, sl, :], AF.Exp,
                         scale=-1.0)
            V.tensor_tensor(q8_all[:, sl, :], qs_all[:, sl, :],
                            rq_all[:, sl, :], ALU.mult)
        for g in range(4):
            for kc in range(FC):
                nc.sync.dma_start_transpose(qt_all[:, g, kc, :],
                                            q8_all[:, g, P * kc:P * kc + P])
        for g in range(4):
            p_o = ps.tile([8, NCLS], F32, tag="sm")
            for o in range(H):
                for kc in range(FC):
                    jc = o * FC + kc
                    TE.matmul(p_o[:],
                              qt_all[:, g, kc, o:P:H],
                              wct_sb[:, jc, :],
                              start=(jc == 0), stop=(jc == H * FC - 1))
            out_f = wk.tile([8, NCLS], F32, tag="outf")
            V.tensor_tensor(out_f[:], p_o[:], bc_rep[:], ALU.add)
            nc.sync.dma_start(out_l[:].rearrange("(g e) n -> g e n", g=4)[g],
                        out_f[:])

    nc.finalize()
    return nc


def kernel(**inputs):
    x = np.asarray(inputs["x"], np.float32)            # [256,1,512]
    nb = np.asarray(inputs["neighbor"], np.float32)    # [256,32,1,512]
    if "tab" not in _CACHE:
        _CACHE["tab"] = build_tables()
    if "prog" not in _CACHE:
        _CACHE["prog"] = build_program()
    nc = _CACHE["prog"]

    shared = {
        "att1": np.ascontiguousarray(
            np.asarray(inputs["att1_w"], np.float32)[None, :]),
        "att2": np.ascontiguousarray(
            np.asarray(inputs["att2_w"], np.float32)[None, :]),
        "tabd": _CACHE["tab"],
        "w1c": np.ascontiguousarray(np.asarray(inputs["W1"], np.float32)),
        "b1": np.asarray(inputs["b1"], np.float32)[:, None].copy(),
        "g1": np.asarray(inputs["g1"], np.float32)[:, None].copy(),
        "be1": np.asarray(inputs["be1"], np.float32)[:, None].copy(),
        "w2": np.ascontiguousarray(np.asarray(inputs["W2"], np.float32)),
        "w2t": np.ascontiguousarray(np.asarray(inputs["W2"],
                                               np.float32).T),
        "b2": np.asarray(inputs["b2"], np.float32)[:, None].copy(),
        "g2": np.asarray(inputs["g2"], np.float32)[:, None].copy(),
        "be2": np.asarray(inputs["be2"], np.float32)[:, None].copy(),
        "wct": np.ascontiguousarray(
            np.asarray(inputs["Wc"], np.float32).T.astype(np.float16)),
        "bc": np.ascontiguousarray(
            np.asarray(inputs["bc"], np.float32)[None, :]),
    }
    in_maps = []
    for c in range(NCORES):
        sl = slice(c * BL, (c + 1) * BL)
        m = dict(shared)
        m["x_l"] = np.ascontiguousarray(x[sl, 0, :])
        m["nb_l"] = np.ascontiguousarray(
            nb[sl, :, 0, :].reshape(BL * N, F))
        in_maps.append(m)

    res = run_bass_kernel_spmd(nc, in_maps, core_ids=list(range(NCORES)))
    return np.concatenate([r["out_l"] for r in res.results], axis=0)
